# revision 17
# baseline (speedup 1.0000x reference)
"""Trainium2 Bass kernel for nn_ConditionalSplineSQ2D.

Math:
  out[b] = sum_{g,h,c} coeffs[g,h,c] * p[b,g,h,ii_c] * p[b,g,h,jj_c]
         = sum_{cells} p_cell^T S_cell p_cell            (S_cell symmetric 8x8)
         = sum_{cells} sum_k lam[cell,k] * (V[cell]^T p_cell)_k^2

Host precomputes the eigendecomposition of the 961 8x8 matrices; the device
kernel per 16-cell group does:
  mm1 (PE): 4 concurrent K=32 diagonal-tile matmuls T = Wblk^T @ P
            (compact per-block [32,32] stationary, fp16 -- 4x less W DMA
             than a dense 128x128 block-diagonal embedding)
  sq  (ACT + DVE + GPSIMD split): Q = T*T   (PSUM -> SBUF bf16)
  mm2 (PE): acc[32j,:] += lam_g^T @ Q_g, clustered one BLOCK behind mm1 so
            consecutive same-shape matmuls pipeline at the streaming floor
            and never wait on a fresh square.

Sharding: pure data parallel over batch (512 per core x 8 cores); the
4 partial accumulator rows per core are summed on host.
"""

import numpy as np

B, G, P = 4096, 31, 8
NCORES = 8
NB = B // NCORES  # 512 batches per core
CELLS = G * G  # 961
GROUP_CELLS = 16
NGROUPS = -(-CELLS // GROUP_CELLS)  # 61
CELLS_PAD = NGROUPS * GROUP_CELLS  # 976
PARTS = 128
TG = 3   # groups per PSUM tile (3 banks x 2 bufs)
NT = -(-NGROUPS // TG)  # 21 psum tiles
ACT_COLS = 1020  # per-tile square columns on ScalarE; rest VectorE copy+mul
N_WARM = 5       # full-array junk matmuls engage the HAM activity monitor
BLOCK = 9        # groups per mm2 cluster (1-block lag behind mm1)
# DMA chunk sizes in groups: sized so queue dispatch (~0.7us per chunk)
# stays ahead of the transfers and the stream never idles
_CHUNKS = [2, 4] + [8] * 6 + [7]
assert sum(_CHUNKS) == NGROUPS and all(c > 0 for c in _CHUNKS)

_nc_cache = {}


def _build_nc():
    import concourse.mybir as mybir
    import concourse.tile as tile
    from concourse import bacc

    nc = bacc.Bacc()
    pt_d = nc.dram_tensor(
        "pt", [PARTS, NGROUPS * NB], mybir.dt.float16, kind="ExternalInput"
    )
    w_d = nc.dram_tensor(
        "wblk", [PARTS, NGROUPS * 32], mybir.dt.float16, kind="ExternalInput"
    )
    lam_d = nc.dram_tensor(
        "lamt", [PARTS, NGROUPS], mybir.dt.bfloat16, kind="ExternalInput"
    )
    out_d = nc.dram_tensor("out", [4, NB], mybir.dt.float32, kind="ExternalOutput")

    with tile.TileContext(nc) as tc:
        with (
            tc.tile_pool(name="const", bufs=1) as cpool,
            tc.tile_pool(name="qp", bufs=8) as qpool,
            tc.tile_pool(name="qcp", bufs=4) as qcpool,
            tc.tile_pool(name="psp", bufs=2, space="PSUM") as pspool,
            tc.tile_pool(name="accp", bufs=1, space="PSUM") as apool,
            tc.tile_pool(name="warmp", bufs=1, space="PSUM") as wpool,
        ):
            w_sb = cpool.tile([PARTS, NGROUPS * 32], mybir.dt.float16)
            lam_sb = cpool.tile([PARTS, NGROUPS], mybir.dt.bfloat16)
            pt_sb = cpool.tile([PARTS, NGROUPS * NB], mybir.dt.float16)
            warm_sb = cpool.tile([PARTS, NB], mybir.dt.float16)
            acc = apool.tile([PARTS, NB], mybir.dt.float32)
            warm_ps = wpool.tile([PARTS, NB], mybir.dt.float32)

            # PE warmup: full-array (K=128, M=128) junk matmuls light up the
            # HAM activity monitor so real work runs at 2.4 GHz from the
            # start; they overlap the DMA ramp and delay nothing.
            nc.gpsimd.memset(warm_sb[:, :], 0.0)
            for _ in range(N_WARM):
                nc.tensor.matmul(
                    warm_ps[:, :], warm_sb[:, :PARTS], warm_sb[:, :],
                    start=True, stop=True,
                )

            # ALL input DMAs on ONE HW queue, in exact consumption order:
            # a second queue steals bandwidth from this one and reorders
            # completions (measured: a 96 KB transfer on a side queue
            # finished 6 us late and stalled the first matmul)
            def chunk_dmas():
                nc.sync.dma_start(out=w_sb[:, : 12 * 32], in_=w_d[:, : 12 * 32])
                g0 = 0
                for k, ch in enumerate(_CHUNKS):
                    nc.sync.dma_start(
                        out=pt_sb[:, g0 * NB : (g0 + ch) * NB],
                        in_=pt_d[:, g0 * NB : (g0 + ch) * NB],
                    )
                    g0 += ch
                    if k == 0:
                        nc.sync.dma_start(out=lam_sb[:, :], in_=lam_d[:, :])
                        nc.sync.dma_start(
                            out=w_sb[:, 12 * 32 :], in_=w_d[:, 12 * 32 :]
                        )

            chunk_dmas()

            q_slices = {}  # group -> (q_tile, slot)
            n_rounds = -(-NGROUPS // 4)  # 16 mm2 rounds of up to 4 groups
            rounds_emitted = 0
            last_round_of_pos = {}  # col pos j -> last round index using it
            for r in range(n_rounds):
                for j in range(4):
                    if r * 4 + j < NGROUPS:
                        last_round_of_pos[j] = r

            def emit_mm2_rounds(limit_group):
                """Emit mm2 rounds whose groups are all squared (< limit)."""
                nonlocal rounds_emitted
                while rounds_emitted < n_rounds:
                    r = rounds_emitted
                    hi = min(r * 4 + 4, NGROUPS)
                    if hi > limit_group:
                        return
                    for j in range(4):
                        g = r * 4 + j
                        if g >= NGROUPS:
                            break
                        qt, slot = q_slices.pop(g)
                        nc.tensor.matmul(
                            acc[32 * j : 32 * j + 1, :],
                            lam_sb[:, g : g + 1],
                            qt[:, slot * NB : (slot + 1) * NB],
                            start=(r == 0),
                            stop=(r == last_round_of_pos[j]),
                            tile_position=(0, 32 * j),
                        )
                    rounds_emitted += 1

            def emit_squares(t):
                # square tile t, split across both engines by columns; runs
                # one tile BEHIND mm1 so ACT/DVE never wait on fresh data
                # and stream back-to-back (they are the steady-state pacer)
                ng = min(TG, NGROUPS - t * TG)
                ncols = ng * NB
                ps = tiles[t]
                q = qpool.tile([PARTS, TG * NB], mybir.dt.bfloat16, tag="q")
                a = (ncols * ACT_COLS) // (TG * NB)
                nc.scalar.square(q[:, :a], ps[:, :a])
                qc = qcpool.tile([PARTS, TG * NB], mybir.dt.bfloat16,
                                 tag="qc")
                nc.vector.tensor_copy(qc[:, a:ncols], ps[:, a:ncols])
                nc.vector.tensor_mul(
                    q[:, a:ncols], qc[:, a:ncols], qc[:, a:ncols]
                )
                for k in range(ng):
                    q_slices[t * TG + k] = (q, k)

            tiles = {}
            for g in range(NGROUPS):
                t, gg = divmod(g, TG)
                if gg == 0:
                    psT = pspool.tile(
                        [PARTS, TG * NB], mybir.dt.float32, tag="psT"
                    )
                    tiles[t] = psT
                # mm1: 4 concurrent K=32 diagonal 32x32 tiles
                for r in range(4):
                    nc.tensor.matmul(
                        tiles[t][32 * r : 32 * r + 32, gg * NB : (gg + 1) * NB],
                        w_sb[32 * r : 32 * r + 32, g * 32 : (g + 1) * 32],
                        pt_sb[32 * r : 32 * r + 32, g * NB : (g + 1) * NB],
                        start=True,
                        stop=True,
                        tile_position=(32 * r, 32 * r),
                    )
                if gg == 0 and t > 0:
                    emit_squares(t - 1)
                # mm2s run in clusters one BLOCK behind mm1 (tapering to the
                # live edge for the final groups so the tail stays short)
                if g >= NGROUPS - 1 - BLOCK and gg == TG - 1:
                    emit_mm2_rounds(g + 1 - 9)
                elif (g + 1) % BLOCK == 0:
                    emit_mm2_rounds(g + 1 - BLOCK)
            emit_squares(NT - 1)
            emit_mm2_rounds(NGROUPS)

            # evacuate the accumulator rows with one wide copy (the unused
            # rows carry junk; the strided DMA below reads only 0/32/64/96)
            out_sb = cpool.tile([PARTS, NB], mybir.dt.float32)
            nc.vector.tensor_copy(out_sb[0:97, :], acc[0:97, :])
            nc.sync.dma_start(out=out_d[0:4, :], in_=out_sb[0:128:32, :])
    if not nc.is_finalized():
        nc.finalize()
    return nc


def _get_nc():
    if "nc" not in _nc_cache:
        _nc_cache["nc"] = _build_nc()
    return _nc_cache["nc"]


def _host_prep_weights(integral_coeffs):
    """coeffs [G,G,C] -> (wblk [128, NGROUPS*32] fp16, lamt [128, NGROUPS] bf16).

    wblk holds, for group g and diagonal tile r, the 32x32 block-diagonal
    stationary lhsT for cells 16g+4r+0..3:
        wblk[32r + 8a + i, 32g + 8a + k] = V[16g+4r+a][i, k]
    """
    ii, jj = np.triu_indices(P)
    w = integral_coeffs.reshape(CELLS, len(ii)).astype(np.float64)
    S = np.zeros((CELLS, P, P), np.float64)
    # quadratic form: off-diag split in half, diag gets full coeff
    np.add.at(S, (slice(None), ii, jj), 0.5 * w)
    np.add.at(S, (slice(None), jj, ii), 0.5 * w)
    lam, V = np.linalg.eigh(S)  # V columns are eigenvectors

    lam_p = np.zeros((CELLS_PAD, P))
    lam_p[:CELLS] = lam
    V_p = np.zeros((CELLS_PAD, P, P))
    V_p[:CELLS] = V

    # [NGROUPS, 4r, 4a, P, P] -> wblk[32r+8a+i, 32g+8a+k]
    Vg = V_p.reshape(NGROUPS, 4, 4, P, P)
    wb = np.zeros((NGROUPS, 4, 4, P, 4, P), np.float32)  # g, r, a, i, a', k
    a = np.arange(4)
    wb[:, :, a, :, a, :] = Vg.transpose(2, 0, 1, 3, 4)
    # order partitions as [r, a, i] and cols as [g, a', k]
    wblk = (
        wb.transpose(1, 2, 3, 0, 4, 5)  # r, a, i, g, a', k
        .reshape(PARTS, NGROUPS * 32)
        .astype(np.float16)
    )
    import jax.numpy as jnp

    lamt = np.asarray(
        jnp.asarray(lam_p.reshape(NGROUPS, PARTS).T, dtype=jnp.bfloat16)
    )
    return np.ascontiguousarray(wblk), np.ascontiguousarray(lamt)


def _host_prep_param(param_tensor):
    """param [B,G,G,P] f32 -> list of per-core [128, NGROUPS*NB] fp16 arrays."""
    flat = param_tensor.reshape(B, CELLS * P)
    out = []
    for c in range(NCORES):
        shard = flat[c * NB : (c + 1) * NB]
        pad = np.zeros((NB, CELLS_PAD * P), np.float32)
        pad[:, : CELLS * P] = shard
        # (b, g, p) -> (p, g, b)
        pt = (
            pad.reshape(NB, NGROUPS, PARTS)
            .transpose(2, 1, 0)
            .reshape(PARTS, NGROUPS * NB)
            .astype(np.float16)
        )
        out.append(np.ascontiguousarray(pt))
    return out


def _run(param_tensor, integral_coeffs, trace=False, **run_kwargs):
    from concourse.bass_utils import run_bass_kernel_spmd

    nc = _get_nc()
    wblk, lamt = _host_prep_weights(np.asarray(integral_coeffs, np.float32))
    pts = _host_prep_param(np.asarray(param_tensor, np.float32))
    in_maps = [{"pt": pts[c], "wblk": wblk, "lamt": lamt} for c in range(NCORES)]
    res = run_bass_kernel_spmd(
        nc, in_maps, core_ids=list(range(NCORES)), trace=trace, **run_kwargs
    )
    out = np.concatenate(
        [res.results[c]["out"].sum(axis=0).reshape(NB) for c in range(NCORES)]
    ).astype(np.float32)
    return out, res


def kernel(param_tensor, integral_coeffs):
    out, _ = _run(param_tensor, integral_coeffs)
    return out


# revision 18
# speedup vs baseline: 1.0471x; 1.0471x over previous
"""Trainium2 Bass kernel for nn_ConditionalSplineSQ2D.

Math:
  out[b] = sum_{g,h,c} coeffs[g,h,c] * p[b,g,h,ii_c] * p[b,g,h,jj_c]
         = sum_{cells} p_cell^T S_cell p_cell            (S_cell symmetric 8x8)
         = sum_{cells} sum_k lam[cell,k] * (V[cell]^T p_cell)_k^2

Host precomputes the eigendecomposition of the 961 8x8 matrices; the device
kernel per 16-cell group does:
  mm1 (PE): 4 concurrent K=32 diagonal-tile matmuls T = Wblk^T @ P
            (compact per-block [32,32] stationary, fp16 -- 4x less W DMA
             than a dense 128x128 block-diagonal embedding)
  sq  (ACT + DVE + GPSIMD split): Q = T*T   (PSUM -> SBUF bf16)
  mm2 (PE): acc[32j,:] += lam_g^T @ Q_g, clustered one BLOCK behind mm1 so
            consecutive same-shape matmuls pipeline at the streaming floor
            and never wait on a fresh square.

Sharding: pure data parallel over batch (512 per core x 8 cores); the
4 partial accumulator rows per core are summed on host.
"""

import numpy as np

B, G, P = 4096, 31, 8
NCORES = 8
NB = B // NCORES  # 512 batches per core
CELLS = G * G  # 961
GROUP_CELLS = 16
NGROUPS = -(-CELLS // GROUP_CELLS)  # 61
CELLS_PAD = NGROUPS * GROUP_CELLS  # 976
PARTS = 128
TG = 3   # groups per PSUM tile (3 banks x 2 bufs)
NT = -(-NGROUPS // TG)  # 21 psum tiles
ACT_COLS = 1020  # per-tile square columns on ScalarE; rest VectorE copy+mul
N_WARM = 8       # full-array junk matmuls: ~4.9us busy spans the full
                 # HAM activity window, so real work starts at 2.4 GHz
BLOCK = 9        # groups per mm2 cluster (1-block lag behind mm1)
# DMA chunk sizes in groups: sized so queue dispatch (~0.7us per chunk)
# stays ahead of the transfers and the stream never idles
_CHUNKS = [2, 4] + [8] * 6 + [7]
assert sum(_CHUNKS) == NGROUPS and all(c > 0 for c in _CHUNKS)

_nc_cache = {}


def _build_nc():
    import concourse.mybir as mybir
    import concourse.tile as tile
    from concourse import bacc

    nc = bacc.Bacc()
    pt_d = nc.dram_tensor(
        "pt", [PARTS, NGROUPS * NB], mybir.dt.float16, kind="ExternalInput"
    )
    w_d = nc.dram_tensor(
        "wblk", [PARTS, NGROUPS * 32], mybir.dt.float16, kind="ExternalInput"
    )
    lam_d = nc.dram_tensor(
        "lamt", [PARTS, NGROUPS], mybir.dt.bfloat16, kind="ExternalInput"
    )
    out_d = nc.dram_tensor("out", [4, NB], mybir.dt.float32, kind="ExternalOutput")

    with tile.TileContext(nc) as tc:
        with (
            tc.tile_pool(name="const", bufs=1) as cpool,
            tc.tile_pool(name="qp", bufs=8) as qpool,
            tc.tile_pool(name="qcp", bufs=4) as qcpool,
            tc.tile_pool(name="psp", bufs=2, space="PSUM") as pspool,
            tc.tile_pool(name="accp", bufs=1, space="PSUM") as apool,
            tc.tile_pool(name="warmp", bufs=1, space="PSUM") as wpool,
        ):
            w_sb = cpool.tile([PARTS, NGROUPS * 32], mybir.dt.float16)
            lam_sb = cpool.tile([PARTS, NGROUPS], mybir.dt.bfloat16)
            pt_sb = cpool.tile([PARTS, NGROUPS * NB], mybir.dt.float16)
            warm_sb = cpool.tile([PARTS, NB], mybir.dt.float16)
            acc = apool.tile([PARTS, NB], mybir.dt.float32)
            warm_ps = wpool.tile([PARTS, NB], mybir.dt.float32)

            # PE warmup: full-array (K=128, M=128) junk matmuls light up the
            # HAM activity monitor so real work runs at 2.4 GHz from the
            # start; they overlap the DMA ramp and delay nothing.
            nc.gpsimd.memset(warm_sb[:, :], 0.0)
            for _ in range(N_WARM):
                nc.tensor.matmul(
                    warm_ps[:, :], warm_sb[:, :PARTS], warm_sb[:, :],
                    start=True, stop=True,
                )

            # ALL input DMAs on ONE HW queue, in exact consumption order:
            # a second queue steals bandwidth from this one and reorders
            # completions (measured: a 96 KB transfer on a side queue
            # finished 6 us late and stalled the first matmul)
            def chunk_dmas():
                nc.sync.dma_start(out=w_sb[:, : 12 * 32], in_=w_d[:, : 12 * 32])
                g0 = 0
                for k, ch in enumerate(_CHUNKS):
                    nc.sync.dma_start(
                        out=pt_sb[:, g0 * NB : (g0 + ch) * NB],
                        in_=pt_d[:, g0 * NB : (g0 + ch) * NB],
                    )
                    g0 += ch
                    if k == 0:
                        nc.sync.dma_start(out=lam_sb[:, :], in_=lam_d[:, :])
                        nc.sync.dma_start(
                            out=w_sb[:, 12 * 32 :], in_=w_d[:, 12 * 32 :]
                        )

            chunk_dmas()

            q_slices = {}  # group -> (q_tile, slot)
            n_rounds = -(-NGROUPS // 4)  # 16 mm2 rounds of up to 4 groups
            rounds_emitted = 0
            last_round_of_pos = {}  # col pos j -> last round index using it
            for r in range(n_rounds):
                for j in range(4):
                    if r * 4 + j < NGROUPS:
                        last_round_of_pos[j] = r

            def emit_mm2_rounds(limit_group):
                """Emit mm2 rounds whose groups are all squared (< limit)."""
                nonlocal rounds_emitted
                while rounds_emitted < n_rounds:
                    r = rounds_emitted
                    hi = min(r * 4 + 4, NGROUPS)
                    if hi > limit_group:
                        return
                    for j in range(4):
                        g = r * 4 + j
                        if g >= NGROUPS:
                            break
                        qt, slot = q_slices.pop(g)
                        nc.tensor.matmul(
                            acc[32 * j : 32 * j + 1, :],
                            lam_sb[:, g : g + 1],
                            qt[:, slot * NB : (slot + 1) * NB],
                            start=(r == 0),
                            stop=(r == last_round_of_pos[j]),
                            tile_position=(0, 32 * j),
                        )
                    rounds_emitted += 1

            def emit_squares(t):
                # square tile t, split across both engines by columns; runs
                # one tile BEHIND mm1 so ACT/DVE never wait on fresh data
                # and stream back-to-back (they are the steady-state pacer)
                ng = min(TG, NGROUPS - t * TG)
                ncols = ng * NB
                ps = tiles[t]
                q = qpool.tile([PARTS, TG * NB], mybir.dt.bfloat16, tag="q")
                a = (ncols * ACT_COLS) // (TG * NB)
                nc.scalar.square(q[:, :a], ps[:, :a])
                qc = qcpool.tile([PARTS, TG * NB], mybir.dt.bfloat16,
                                 tag="qc")
                nc.vector.tensor_copy(qc[:, a:ncols], ps[:, a:ncols])
                nc.vector.tensor_mul(
                    q[:, a:ncols], qc[:, a:ncols], qc[:, a:ncols]
                )
                for k in range(ng):
                    q_slices[t * TG + k] = (q, k)

            tiles = {}
            for g in range(NGROUPS):
                t, gg = divmod(g, TG)
                if gg == 0:
                    psT = pspool.tile(
                        [PARTS, TG * NB], mybir.dt.float32, tag="psT"
                    )
                    tiles[t] = psT
                # mm1: 4 concurrent K=32 diagonal 32x32 tiles
                for r in range(4):
                    nc.tensor.matmul(
                        tiles[t][32 * r : 32 * r + 32, gg * NB : (gg + 1) * NB],
                        w_sb[32 * r : 32 * r + 32, g * 32 : (g + 1) * 32],
                        pt_sb[32 * r : 32 * r + 32, g * NB : (g + 1) * NB],
                        start=True,
                        stop=True,
                        tile_position=(32 * r, 32 * r),
                    )
                if gg == TG - 1 or g == NGROUPS - 1:
                    emit_squares(t)
                # mm2s run in clusters one BLOCK behind mm1 (tapering to the
                # live edge for the final groups so the tail stays short)
                if g >= NGROUPS - 1 - BLOCK and gg == TG - 1:
                    emit_mm2_rounds(g + 1 - 9)
                elif (g + 1) % BLOCK == 0:
                    emit_mm2_rounds(g + 1 - BLOCK)
            emit_mm2_rounds(NGROUPS)

            # evacuate the accumulator rows with one wide copy (the unused
            # rows carry junk; the strided DMA below reads only 0/32/64/96)
            out_sb = cpool.tile([PARTS, NB], mybir.dt.float32)
            nc.vector.tensor_copy(out_sb[0:97, :], acc[0:97, :])
            nc.sync.dma_start(out=out_d[0:4, :], in_=out_sb[0:128:32, :])
    if not nc.is_finalized():
        nc.finalize()
    return nc


def _get_nc():
    if "nc" not in _nc_cache:
        _nc_cache["nc"] = _build_nc()
    return _nc_cache["nc"]


def _host_prep_weights(integral_coeffs):
    """coeffs [G,G,C] -> (wblk [128, NGROUPS*32] fp16, lamt [128, NGROUPS] bf16).

    wblk holds, for group g and diagonal tile r, the 32x32 block-diagonal
    stationary lhsT for cells 16g+4r+0..3:
        wblk[32r + 8a + i, 32g + 8a + k] = V[16g+4r+a][i, k]
    """
    ii, jj = np.triu_indices(P)
    w = integral_coeffs.reshape(CELLS, len(ii)).astype(np.float64)
    S = np.zeros((CELLS, P, P), np.float64)
    # quadratic form: off-diag split in half, diag gets full coeff
    np.add.at(S, (slice(None), ii, jj), 0.5 * w)
    np.add.at(S, (slice(None), jj, ii), 0.5 * w)
    lam, V = np.linalg.eigh(S)  # V columns are eigenvectors

    lam_p = np.zeros((CELLS_PAD, P))
    lam_p[:CELLS] = lam
    V_p = np.zeros((CELLS_PAD, P, P))
    V_p[:CELLS] = V

    # [NGROUPS, 4r, 4a, P, P] -> wblk[32r+8a+i, 32g+8a+k]
    Vg = V_p.reshape(NGROUPS, 4, 4, P, P)
    wb = np.zeros((NGROUPS, 4, 4, P, 4, P), np.float32)  # g, r, a, i, a', k
    a = np.arange(4)
    wb[:, :, a, :, a, :] = Vg.transpose(2, 0, 1, 3, 4)
    # order partitions as [r, a, i] and cols as [g, a', k]
    wblk = (
        wb.transpose(1, 2, 3, 0, 4, 5)  # r, a, i, g, a', k
        .reshape(PARTS, NGROUPS * 32)
        .astype(np.float16)
    )
    import jax.numpy as jnp

    lamt = np.asarray(
        jnp.asarray(lam_p.reshape(NGROUPS, PARTS).T, dtype=jnp.bfloat16)
    )
    return np.ascontiguousarray(wblk), np.ascontiguousarray(lamt)


def _host_prep_param(param_tensor):
    """param [B,G,G,P] f32 -> list of per-core [128, NGROUPS*NB] fp16 arrays."""
    flat = param_tensor.reshape(B, CELLS * P)
    out = []
    for c in range(NCORES):
        shard = flat[c * NB : (c + 1) * NB]
        pad = np.zeros((NB, CELLS_PAD * P), np.float32)
        pad[:, : CELLS * P] = shard
        # (b, g, p) -> (p, g, b)
        pt = (
            pad.reshape(NB, NGROUPS, PARTS)
            .transpose(2, 1, 0)
            .reshape(PARTS, NGROUPS * NB)
            .astype(np.float16)
        )
        out.append(np.ascontiguousarray(pt))
    return out


def _run(param_tensor, integral_coeffs, trace=False, **run_kwargs):
    from concourse.bass_utils import run_bass_kernel_spmd

    nc = _get_nc()
    wblk, lamt = _host_prep_weights(np.asarray(integral_coeffs, np.float32))
    pts = _host_prep_param(np.asarray(param_tensor, np.float32))
    in_maps = [{"pt": pts[c], "wblk": wblk, "lamt": lamt} for c in range(NCORES)]
    res = run_bass_kernel_spmd(
        nc, in_maps, core_ids=list(range(NCORES)), trace=trace, **run_kwargs
    )
    out = np.concatenate(
        [res.results[c]["out"].sum(axis=0).reshape(NB) for c in range(NCORES)]
    ).astype(np.float32)
    return out, res


def kernel(param_tensor, integral_coeffs):
    out, _ = _run(param_tensor, integral_coeffs)
    return out
